# revision 80
# baseline (speedup 1.0000x reference)
"""Multi-head(1) attention kernel for Trainium2, 8 NeuronCores SPMD.

Problem: x[4,4096,1024] @ {Wq,Wk,Wv}[1024,128] -> q,k,v; softmax(q k^T/sqrt(128)) v.

Sharding: core c -> (batch b = c//2, query-half h = c%2).
Each core receives xT = x[b].T (d_model on rows) with the 4096 columns permuted
so that "my" 2048 query rows come first. The core computes kT/v for all 4096
keys (key order is irrelevant under softmax), qT for its first 2048 columns,
and emits outT [128, 2048] = (attention output for its query rows)^T.

v3 (fused): all matmul operands bfloat16 (host converts x and W; halves HBM
traffic, keeps PE at 1 cyc/row, unlocks DVE 2x adds).  The projection sweep
and the attention of the first NFQ=3 query blocks are FUSED into one pass:
query block qq processes key block (n - qq) during round n (the lag keeps
every operand one round old, so nothing in the attention stream waits on
same-round projections).  Projection matmuls for round n+1 are emitted as
fine-grained "background ops" interleaved between attention chunk-iters so
the in-order PE stream always has non-dependent work while ACT exps run.
V is projected directly in natural [key, dv] layout (x-slice stationary,
8x128-row accumulation) -- no transposes, no transpose PSUM pool.  The 4th
query block runs as a short pass 2 (paired [128,1024] exps) that is
ACT-bound; its PE work fits in the tail.

Engine budget (per core, 2.4GHz PE / 1.2GHz ACT / 0.96GHz DVE):
  pass 1: PE ~183k cyc (proj 82k + scores/PV 98k + epilogues) ~77us;
          ACT 96 exps x 612ns ~59us; DVE (evacs + bf16 denominator) ~55us.
  pass 2: ACT-bound 16x1038ns exps ~17us; PE 14us.  Denominators:
  per-key-block bf16 fold trees + sequential accumulate on DVE; partition
  sum via a ones-vector matmul; normalization broadcast via a ones-row
  matmul; final multiply reads PSUM directly on DVE.

PSUM (8 banks x 2KB/partition): po_0..2 (3) + proj ring kqv (2) + scores
ring (3); epilogue dps/bps and absorber scratch ride the rings.  Pass 2
swaps the pass-1 rings for a [128,1024] pair ring (4) keeping po (3).

Absorber matmuls (tiny PE reads of DMA'd/POOL-produced tiles) keep real
matmuls at <=1 sync wait (LDWEIGHTS can encode only one).

Measured: ~136-137us HW (timeloop steady-state) vs 188us f32r 2-phase
baseline; TimelineSim 115us; rel err vs f32 reference 4.2e-3 (gate 2e-2).
Startup DMAs are interleaved (wk, x0 parts, wq, ...) for the one-shot cold
start, and outT is bf16 (host casts back).  Note: host-pre-permuting W to
make its DMA contiguous measured ~1us better in TimelineSim but ~12us
WORSE on hardware (twice) -- do not reintroduce.
"""

import math

import numpy as np

import concourse.bacc as bacc
import concourse.bass as bass
import concourse.mybir as mybir
from concourse.bass import ts
from concourse.tile import TileContext

P = 128
D_MODEL = 1024
D_QK = 128
B = 4
S_FULL = 4096
N_CORES = 8

F32 = mybir.dt.float32
F32R = mybir.dt.float32r
BF16 = mybir.dt.bfloat16
AF = mybir.ActivationFunctionType

SM_SCALE = 1.0 / math.sqrt(D_QK)

MM_DT = BF16


def build_attention(nc: bass.Bass, S: int = S_FULL, SQ: int = S_FULL // 2, repeat: int = 1):
    """Emit the SPMD single-core program. S = #keys, SQ = #queries."""
    assert S % 512 == 0 and SQ % 512 == 0 and D_MODEL % P == 0
    DC = D_MODEL // P  # 8 d_model chunks
    NB = S // 512  # xT column blocks
    QNB = SQ // 512  # of which query blocks
    KC = S // P  # 32 k chunks
    KP = KC // 2  # k-chunk pairs (pass 2)
    NFQ = min(3, QNB)  # query blocks fused into the projection sweep

    xT = nc.dram_tensor("xT", [D_MODEL, S], MM_DT, kind="ExternalInput").ap()
    wq = nc.dram_tensor("Wq", [D_MODEL, D_QK], MM_DT, kind="ExternalInput").ap()
    bq = nc.dram_tensor("bq", [D_QK], F32, kind="ExternalInput").ap()
    wk = nc.dram_tensor("Wk", [D_MODEL, D_QK], MM_DT, kind="ExternalInput").ap()
    bk = nc.dram_tensor("bk", [D_QK], F32, kind="ExternalInput").ap()
    wv = nc.dram_tensor("Wv", [D_MODEL, D_QK], MM_DT, kind="ExternalInput").ap()
    bv = nc.dram_tensor("bv", [D_QK], F32, kind="ExternalInput").ap()
    outT = nc.dram_tensor("outT", [D_QK, SQ], BF16, kind="ExternalOutput").ap()

    with TileContext(nc) as tc:
        lp = nc.allow_low_precision(reason="bf16 matmuls + bf16 denominator accum")
        lp.__enter__()
        if repeat > 1:
            loop_cm = tc.For_i(0, repeat, 1)
            loop_cm.__enter__()
        with (
            tc.tile_pool(name="persist", bufs=1) as pp,
            tc.tile_pool(name="xt_pool", bufs=3) as xp,
            tc.tile_pool(name="wka", bufs=3) as wka,
            tc.tile_pool(name="u_pool", bufs=8) as up,
            tc.tile_pool(name="fold_pool", bufs=2) as fp,
            tc.tile_pool(name="acc_pool", bufs=1) as accp,
            tc.tile_pool(name="wkb", bufs=3) as wkb,
            tc.tile_pool(name="poB", bufs=1, space="PSUM") as poB,
        ):
            # --- constants (weight/bias DMAs issued later, in an order that
            # minimizes the first projection matmul's start time) ---
            w_sb = {}
            w_src = {"q": wq, "k": wk, "v": wv}
            for nm in ("q", "k", "v"):
                t = pp.tile([P, DC * D_QK], MM_DT, tag=f"w{nm}", name=f"w{nm}_sb")
                w_sb[nm] = t

            def dma_w(nm):
                nc.sync.dma_start(
                    out=w_sb[nm].rearrange("p (c n) -> p c n", n=D_QK),
                    in_=w_src[nm].rearrange("(c p) n -> p c n", p=P),
                )

            b_sb = {}
            b_src = {"q": bq, "k": bk, "v": bv}
            for nm in ("q", "k", "v"):
                t = pp.tile([P, 1], F32, tag=f"b{nm}", name=f"b{nm}_sb")
                b_sb[nm] = t
            ones_col = pp.tile([P, 1], F32, tag="ones_col")
            nc.gpsimd.memset(ones_col, 1.0)
            ones_col_b = pp.tile([P, 1], MM_DT, tag="ones_col_b")
            nc.vector.tensor_copy(out=ones_col_b, in_=ones_col)
            ones_row = pp.tile([1, P], F32, tag="ones_row")
            nc.gpsimd.memset(ones_row, 1.0)
            ones_row_r = pp.tile([1, P], F32R, tag="ones_row_r")
            nc.vector.tensor_copy(out=ones_row_r, in_=ones_row)

            kT = pp.tile([P, S], MM_DT, tag="kT")
            vn = pp.tile([P, S], MM_DT, tag="vn")
            qT = pp.tile([P, SQ], MM_DT, tag="qT")
            vb = pp.tile([P, P], F32, tag="vb")  # bv broadcast across partitions
            bv_row = pp.tile([1, P], F32, tag="bv_row")

            babs = wka.tile([P, 1], F32, tag="babs")

            # --- pass 1: fused projection sweep + attention of qq < NFQ ---
            with (
                tc.tile_pool(name="psA", bufs=2, space="PSUM") as psA,
                tc.tile_pool(name="sps1", bufs=3, space="PSUM") as sp1,
            ):
                scrn = [0]
                absorb_alloc = [
                    lambda name: psA.tile([1, 1], F32, tag="kqv", name=name)
                ]

                def pe_absorb(ap):
                    # tiny PE matmul reading `ap` into a fresh ring slot so
                    # later real matmuls carry at most one sync wait.
                    scrn[0] += 1
                    s = absorb_alloc[0](f"scr_{scrn[0]}")
                    a = ap[:, 0:1]
                    if a.dtype not in (F32, BF16):
                        a = a.bitcast(F32)
                    nc.tensor.matmul(s, a, a, start=True, stop=True)

                xts: dict[int, object] = {}

                def xt_part_dma(n, i, parts=4):
                    # split so early projection matmuls can start as soon as
                    # their d_model chunks land (subtile deps); each consumer
                    # waits on exactly one queue sem.
                    xt3 = xts[n].rearrange("p (c s) -> p c s", s=512)
                    xT3 = xT[:, ts(n, 512)].rearrange("(c p) s -> p c s", p=P)
                    w = DC // parts
                    nc.sync.dma_start(
                        out=xt3[:, i * w : (i + 1) * w],
                        in_=xT3[:, i * w : (i + 1) * w],
                    )

                def emit_xt_dma(n, parts=4):
                    xts[n] = xp.tile(
                        [P, DC * 512], MM_DT, tag="xt", name=f"xt_{n}"
                    )
                    for i in range(parts):
                        xt_part_dma(n, i, parts)

                def proj_ops(n):
                    """Background closures emitting projections for block n
                    (consumed by attention one round later)."""
                    ops = []
                    if n + 2 < NB:
                        # prefetch two blocks ahead (xt ring is 3 deep; this
                        # block's closures run one round before consumption)
                        ops.append(lambda: emit_xt_dma(n + 2))
                    ops.append(lambda: pe_absorb(xts[n]))
                    state: dict = {}

                    def mk_proj(nm, c, first, last, dest):
                        def _op():
                            if first:
                                state[nm] = psA.tile(
                                    [P, 512], F32, tag="kqv", name=f"{nm}ps_{n}"
                                )
                            nc.tensor.matmul(
                                state[nm],
                                w_sb[nm][:, ts(c, D_QK)],
                                xts[n][:, ts(c, 512)],
                                start=first,
                                stop=last,
                            )
                            if last and dest is not None:
                                nc.vector.tensor_scalar_add(
                                    dest, state[nm], b_sb[nm]
                                )

                        return _op

                    for c in range(DC):
                        ops.append(
                            mk_proj("k", c, c == 0, c == DC - 1, kT[:, ts(n, 512)])
                        )
                    if n < QNB:
                        for c in range(DC):
                            ops.append(
                                mk_proj("q", c, c == 0, c == DC - 1, qT[:, ts(n, 512)])
                            )
                    # natural-layout v: out[key, dv] accumulated over d_model
                    # chunks with the x slice as the stationary operand; no
                    # transposes, no extra PSUM pool.  128-row matmuls, two
                    # per background op.
                    xt3v = None

                    def mk_vproj(j, cc):
                        def _op():
                            nonlocal xt3v
                            key = f"v{j}"
                            if cc == 0:
                                state[key] = psA.tile(
                                    [P, P], F32, tag="kqv", name=f"vps_{n}_{j}"
                                )
                            vps = state[key]
                            for c in (cc, cc + 1):
                                nc.tensor.matmul(
                                    vps,
                                    xts[n][:, c * 512 + j * P : c * 512 + (j + 1) * P],
                                    w_sb["v"][:, ts(c, D_QK)],
                                    start=(c == 0),
                                    stop=(c == DC - 1),
                                )
                            if cc + 2 == DC:
                                nc.vector.tensor_add(
                                    out=vn[:, ts(4 * n + j, P)], in0=vps, in1=vb
                                )

                        return _op

                    for j in range(4):
                        for cc in range(0, DC, 2):
                            ops.append(mk_vproj(j, cc))
                    return ops

                accs: dict[int, object] = {}
                pend_pv: list = []
                PVDEPTH = 4

                def emit_pv(qq, ck, u1):
                    nc.tensor.matmul(
                        pos[qq],
                        vn[:, ts(ck, P)],
                        u1,
                        start=(ck == 0),
                        stop=(ck == KC - 1),
                    )

                pos = {
                    qq: poB.tile([P, 512], F32, tag=f"po_{qq}", name=f"po_{qq}")
                    for qq in range(NFQ)
                }

                def epilogue_ops(qq, po, ring, halves):
                    """Closures: softmax denominator -> normalize -> DMA out.
                    `ring` supplies PSUM tiles (pass-1: sps1, pass-2: psB);
                    `halves` lazily yields the two [128,512] bf16 partial-sum
                    tiles whose partition+pair sum is the denominator."""
                    st: dict = {}

                    def e_abs():
                        pe_absorb(halves()[0])

                    def e_dps():
                        st["dps"] = ring([1, 512], F32, f"dps_{qq}")
                        h = halves()
                        nc.tensor.matmul(
                            st["dps"], ones_col_b, h[0], start=True, stop=False
                        )
                        nc.tensor.matmul(
                            st["dps"], ones_col_b, h[1], start=False, stop=True
                        )

                    def e_rec():
                        # po evacuation overlaps the PE's dps/bps matmuls
                        st["poc"] = wkb.tile([P, 512], F32, tag="poc", name=f"poc_{qq}")
                        nc.vector.tensor_copy(out=st["poc"], in_=po)
                        st["rec"] = wkb.tile([1, 512], F32R, tag="rec", name=f"rec_{qq}")
                        nc.vector.reciprocal(out=st["rec"], in_=st["dps"])

                    def e_bps():
                        st["bps"] = ring([P, 512], F32, f"bps_{qq}")
                        nc.tensor.matmul(
                            st["bps"], ones_row_r, st["rec"], start=True, stop=True
                        )

                    def e_fin():
                        # bps is read straight out of PSUM; the one absorber
                        # transitively clears every epilogue WAR (fin waits
                        # rec/poc which wait dps/po)
                        fin = wkb.tile([P, 512], BF16, tag="fin", name=f"fin_{qq}")
                        nc.vector.tensor_mul(out=fin, in0=st["poc"], in1=st["bps"])
                        nc.sync.dma_start(out=outT[:, ts(qq, 512)], in_=fin)
                        pe_absorb(fin)

                    return [e_abs, e_dps, e_rec, e_bps, e_fin]

                def sps_ring(shape, dtype, name):
                    return sp1.tile(shape, dtype, tag="ps", name=name)

                # late-bound ring for epilogues that may spill into pass 2
                # (after the pass-1 PSUM pools close)
                ring_cell = [sps_ring]

                def late_ring(shape, dtype, name):
                    return ring_cell[0](shape, dtype, name)

                # startup DMA queue order: wk/wq, then x block 0, then the
                # small bias tensors, then wv and x block 1 -- so the first
                # kproj matmul starts as soon as possible and nothing the
                # early PE/ACT stream waits on sits behind a bulk transfer.
                xts[0] = xp.tile([P, DC * 512], MM_DT, tag="xt", name="xt_0")
                dma_w("k")
                xt_part_dma(0, 0)
                xt_part_dma(0, 1)
                dma_w("q")
                xt_part_dma(0, 2)
                xt_part_dma(0, 3)
                nc.sync.dma_start(out=bv_row, in_=bv.unsqueeze(0))
                for nm in ("k", "q", "v"):
                    nc.sync.dma_start(out=b_sb[nm], in_=b_src[nm].unsqueeze(1))
                dma_w("v")
                if NB > 1:
                    emit_xt_dma(1, parts=4)

                for nm in ("k", "q"):
                    pe_absorb(w_sb[nm])
                    nc.scalar.copy(out=babs, in_=b_sb[nm])
                pe_absorb(ones_col)
                pe_absorb(ones_col_b)
                pe_absorb(ones_row[0:1, 0:1].broadcast_to([1, 1]))
                pe_absorb(ones_row_r[0:1, 0:1].broadcast_to([1, 1]))

                # prologue: k/q projections for block 0 inline; block 0's v
                # projections (and everything wv-dependent) spill into round
                # 0's background so ACT/DVE start as soon as kT/qT block 0
                # exist
                ops0 = proj_ops(0)

                def v_prep():
                    pe_absorb(w_sb["v"])
                    nc.scalar.copy(out=babs, in_=b_sb["v"])
                    # vb = ones ^T bv (bias along the free dim of natural-v)
                    vbps = psA.tile([P, P], F32, tag="kqv", name="vbps")
                    nc.tensor.matmul(vbps, ones_row, bv_row, start=True, stop=True)
                    nc.vector.tensor_copy(out=vb, in_=vbps)

                v_spill = [v_prep] + ops0[-16:]
                for op in ops0[:-16]:
                    op()

                n_rounds = NB + NFQ - 1
                for n in range(n_rounds):
                    bg = []
                    if n == 0:
                        bg += v_spill
                    if n + 1 < NB:
                        bg += proj_ops(n + 1)
                    for qq in range(NFQ):
                        if n == NB + qq:
                            bg += epilogue_ops(
                                qq, pos[qq], sps_ring, lambda qq=qq: accs[qq]
                            )
                    items = [
                        (qq, n - qq)
                        for qq in range(NFQ)
                        if 0 <= n - qq < NB
                    ]
                    n_iters = 4 * len(items)
                    it = 0
                    for qq, blk in items:
                        ust: dict = {}
                        for j in range(4):
                            # interleave background (projection/epilogue) ops
                            want = (len(bg) * (it + 1)) // n_iters
                            done = (len(bg) * it) // n_iters
                            for _ in range(want - done):
                                bg_op = bg[done]
                                done += 1
                                bg_op()
                            it += 1
                            ck = 4 * blk + j
                            sp = sp1.tile(
                                [P, 512], F32, tag="ps", name=f"sp_{qq}_{ck}"
                            )
                            nc.tensor.matmul(
                                sp,
                                kT[:, ts(ck, P)],
                                qT[:, ts(qq, 512)],
                                start=True,
                                stop=True,
                            )
                            u1 = up.tile(
                                [P, 512], MM_DT, tag="u1", name=f"u_{qq}_{ck}"
                            )
                            nc.scalar.activation(u1, sp, AF.Exp, scale=SM_SCALE)
                            ust[j] = u1
                            if j == 1:
                                fa = fp.tile([P, 512], MM_DT, tag="fA", name=f"fA_{qq}_{blk}")
                                nc.vector.tensor_add(out=fa, in0=ust[0], in1=ust[1])
                                ust["fa"] = fa
                            if j == 3:
                                fb = fp.tile([P, 512], MM_DT, tag="fB", name=f"fB_{qq}_{blk}")
                                nc.vector.tensor_add(out=fb, in0=ust[2], in1=ust[3])
                                fc = fp.tile([P, 512], MM_DT, tag="fC", name=f"fC_{qq}_{blk}")
                                nc.vector.tensor_add(out=fc, in0=ust["fa"], in1=fb)
                                if blk == 0:
                                    acc = accp.tile(
                                        [P, 512], MM_DT, tag=f"acc_{qq}",
                                        name=f"acc_{qq}",
                                    )
                                    accs[qq] = (acc,)
                                    nc.vector.tensor_copy(out=acc, in_=fc)
                                elif blk == NB - 1:
                                    # final add goes to a second tile so the
                                    # epilogue's dacc fold has two operands
                                    accs[qq] = (accs[qq][0], fc)
                                else:
                                    nc.vector.tensor_add(
                                        out=accs[qq][0], in0=accs[qq][0], in1=fc
                                    )
                            pend_pv.append((qq, ck, u1))
                            if len(pend_pv) > PVDEPTH:
                                emit_pv(*pend_pv.pop(0))
                    if n_iters == 0:  # defensive: rounds with no att items
                        for bg_op in bg:
                            bg_op()
                while pend_pv:
                    emit_pv(*pend_pv.pop(0))
                # epilogues not emitted inside rounds spill into pass 2 (or,
                # if there is no pass 2, run here)
                spill: list = []
                for qq in range(NFQ):
                    if NB + qq > n_rounds - 1:
                        spill += epilogue_ops(
                            qq, pos[qq], late_ring, lambda qq=qq: accs[qq]
                        )
                if QNB == NFQ:
                    for op in spill:
                        op()
                    spill = []

            # --- pass 2: remaining query blocks, paired-exp pipeline ---
            if QNB > NFQ:
                with (
                    tc.tile_pool(name="psB", bufs=2, space="PSUM") as psB,
                    # the 8th bank is free in pass 2: dedicate it to the
                    # epilogue dps/bps and absorber scratch so the tail does
                    # not cycle the exp pair-ring
                    tc.tile_pool(name="psE", bufs=1, space="PSUM") as psE,
                ):

                    def psb_ring(shape, dtype, name):
                        return psB.tile(shape, dtype, tag="ps", name=name)

                    def pse_ring(shape, dtype, name):
                        return psE.tile(shape, dtype, tag="pse", name=name)

                    absorb_alloc[0] = lambda name: pse_ring([1, 1], F32, name)
                    ring_cell[0] = pse_ring

                    for qb in range(NFQ, QNB):
                        bg2 = spill
                        spill = []
                        po = poB.tile(
                            [P, 512], F32, tag=f"po_{qb % NFQ}", name=f"po2_{qb}"
                        )
                        us: dict[int, object] = {}
                        dacc2 = accp.tile(
                            [P, 1024], MM_DT, tag="dacc2", name=f"dacc2_{qb}"
                        )
                        for mp in range(KP + 2):
                            if bg2:
                                bg2.pop(0)()
                            if mp < KP:
                                sps = psB.tile(
                                    [P, 1024], F32, tag="ps", name=f"sps_{qb}_{mp}"
                                )
                                for h in range(2):
                                    nc.tensor.matmul(
                                        sps[:, ts(h, 512)],
                                        kT[:, ts(2 * mp + h, P)],
                                        qT[:, ts(qb, 512)],
                                        start=True,
                                        stop=True,
                                    )
                                u = up.tile([P, 1024], MM_DT, tag="u", name=f"u2_{qb}_{mp}")
                                nc.scalar.activation(u, sps, AF.Exp, scale=SM_SCALE)
                                us[mp] = u
                                if mp == 0:
                                    nc.vector.tensor_copy(out=dacc2, in_=u)
                                else:
                                    nc.vector.tensor_add(out=dacc2, in0=dacc2, in1=u)
                            if mp > 1:
                                # 2-pair PV deferral: PV never waits its exp
                                u_prev = us.pop(mp - 2)
                                for h in range(2):
                                    mm = 2 * (mp - 2) + h
                                    nc.tensor.matmul(
                                        po,
                                        vn[:, ts(mm, P)],
                                        u_prev[:, ts(h, 512)],
                                        start=(mm == 0),
                                        stop=(mm == KC - 1),
                                    )
                        for op in epilogue_ops(
                            qb, po, pse_ring,
                            lambda d=dacc2: (d[:, ts(0, 512)], d[:, ts(1, 512)]),
                        ):
                            op()

        if repeat > 1:
            loop_cm.__exit__(None, None, None)

    return nc


_NC_CACHE: dict = {}


def _get_nc(S: int = S_FULL, SQ: int = S_FULL // 2, repeat: int = 1):
    key = (S, SQ, repeat)
    if key not in _NC_CACHE:
        nc = bacc.Bacc("TRN2", debug=False)
        build_attention(nc, S, SQ, repeat)
        nc.compile()  # splits multi-waits into event semaphores (HW limit)
        _NC_CACHE[key] = nc
    return _NC_CACHE[key]


def _bf16(a):
    import ml_dtypes

    return np.ascontiguousarray(np.asarray(a, dtype=np.float32).astype(ml_dtypes.bfloat16))


def make_in_maps(x, Wq, bq, Wk, bk, Wv, bv):
    """Per-core input dicts. Core c = (batch c//2, query-half c%2)."""
    x = np.asarray(x, dtype=np.float32)
    common = {
        "Wq": _bf16(Wq),
        "bq": np.ascontiguousarray(bq, dtype=np.float32),
        "Wk": _bf16(Wk),
        "bk": np.ascontiguousarray(bk, dtype=np.float32),
        "Wv": _bf16(Wv),
        "bv": np.ascontiguousarray(bv, dtype=np.float32),
    }
    in_maps = []
    for c in range(N_CORES):
        b, h = divmod(c, 2)
        xb = x[b]  # [S, D]
        half = S_FULL // 2
        if h == 0:
            perm = xb
        else:
            perm = np.concatenate([xb[half:], xb[:half]], axis=0)
        in_maps.append({"xT": _bf16(perm.T), **common})
    return in_maps


def assemble_output(results):
    """results: list of 8 per-core dicts with 'outT' [128, 2048]."""
    half = S_FULL // 2
    out = np.empty((B, S_FULL, D_QK), dtype=np.float32)
    for c in range(N_CORES):
        b, h = divmod(c, 2)
        out[b, h * half : (h + 1) * half, :] = np.asarray(
            results[c]["outT"], dtype=np.float32
        ).T
    return out


def kernel(x, Wq, bq, Wk, bk, Wv, bv):
    from concourse.bass_utils import run_bass_kernel_spmd

    nc = _get_nc()
    in_maps = make_in_maps(x, Wq, bq, Wk, bk, Wv, bv)
    res = run_bass_kernel_spmd(nc, in_maps, list(range(N_CORES)))
    return assemble_output(res.results)
